# revision 21
# baseline (speedup 1.0000x reference)
"""GridSample (bilinear, zeros padding, align_corners=False, snap-to-ceil quirk)
for Trainium2 via Bass. Batch dim (8) sharded across 8 NeuronCores.

Inputs:  inputs [8,256,256,64] f32, grid [8,256,256,2] f32 in [-1,1)
Output:  [8,256,256,64] f32

Per-core strategy (v2, block-table):
  - Host pre-builds a 2x2-block table: blocktab[y*256+x] = the 4 pixels
    [(y,x),(y,x+1),(y+1,x),(y+1,x+1)] of the zero-padded 257x257 image,
    i.e. one contiguous 1KB row per (y1,x1) bilinear footprint. Every
    output pixel then needs exactly ONE indirect-DMA gather of 1KB
    instead of two 512B gathers (top/bottom pair) — halving the SWDGE
    instruction count, which is the serial bottleneck on GPSIMD
    (~1.1us fixed cost per indirect DMA instruction, 128 descriptors).
  - Device computes the reference's coordinate transform bit-exactly
    (f32 op-for-op), remaps out-of-image taps onto in-bounds block
    slots by clamping (by=max(y1,0), bx=max(x1,0)) and reassigning the
    per-slot weights, so no front padding and no index interleave.
  - Weighted 4-slot sum on DVE/ACT, output streamed back per tile.
"""

import os
import sys

import numpy as np

for _p in ("/opt/trn_rl_repo",):
    if _p not in sys.path and os.path.isdir(_p):
        sys.path.insert(0, _p)

from concourse import bass, mybir
from concourse.bass import IndirectOffsetOnAxis
from concourse.bass_utils import run_bass_kernel_spmd
from concourse.tile import TileContext

# ---------------------------------------------------------------------------
# The walrus build in this container rejects instructions carrying more than
# one sync wait ("Too many sync wait commands", CoreV3GenImpl setupSyncWait).
# TileContext emits multi-wait instructions (e.g. the tail Drain), so split
# every excess wait into its own single-wait EventSemaphore right before the
# owning instruction — semantically identical (engine executes in order).
# ---------------------------------------------------------------------------
import json as _json

from concourse import bass_utils as _bass_utils
from concourse import bass2jax as _bass2jax

_orig_compile_bir_kernel = _bass_utils.compile_bir_kernel


def _split_multiwaits_json(bir_bytes):
    m = _json.loads(bir_bytes)
    changed = False
    for fn in m.get("functions", []):
        for bb in fn.get("blocks", []):
            insts = bb.get("instructions", [])
            out = []
            for ins in insts:
                si = ins.get("sync_info") or {}
                ow = si.get("on_wait") or []
                if len(ow) > 1:
                    changed = True
                    for j, w in enumerate(ow[:-1]):
                        out.append({
                            "debug": ins.get("debug", 0),
                            "engine": ins["engine"],
                            "ins": [],
                            "outs": [],
                            "name": f"{ins['name']}-ws{j}",
                            "opcode": "EventSemaphore",
                            "sync_info": {"on_update": [], "on_wait": [w]},
                        })
                    si["on_wait"] = [ow[-1]]
                out.append(ins)
            bb["instructions"] = out
    if not changed:
        return bir_bytes
    return _json.dumps(m).encode()


def _patched_compile_bir_kernel(bir_json, tmpdir, neff_name="file.neff"):
    return _orig_compile_bir_kernel(
        _split_multiwaits_json(bir_json), tmpdir, neff_name=neff_name)


_bass_utils.compile_bir_kernel = _patched_compile_bir_kernel
_bass2jax.compile_bir_kernel = _patched_compile_bir_kernel

F32 = mybir.dt.float32
BF16 = mybir.dt.bfloat16
I32 = mybir.dt.int32

N, H, W, C = 8, 256, 256, 64
NPX = H * W            # 65536 pixels per image
PPX = NPX // 128       # 512 pixels per partition
NBLK = H * W           # one 2x2 block per (y1,x1) in [0,255]^2
BLKC = 4 * C           # 256 f32 per block row (1KB)
NT = 16                # tiles per core
K = PPX // NT          # 32 pixels per partition per tile
MAGIC = 12582912.0     # 1.5*2^23: x+MAGIC lands in [2^23,2^24) (ulp=1) for all
                       # x in [-1,256], so x + MAGIC - MAGIC == rne(x) exactly

_CACHE = {}


def _build_program():
    nc = bass.Bass()
    table = nc.declare_dram_parameter("table", [NBLK, BLKC], F32, isOutput=False)
    grid_d = nc.declare_dram_parameter("grid", [128, PPX * 2], F32, isOutput=False)
    out_d = nc.declare_dram_parameter("out", [128, PPX * C], F32, isOutput=True)

    with TileContext(nc) as tc:
        with (
            tc.tile_pool(name="const", bufs=1) as cpool,
            tc.tile_pool(name="coord", bufs=1) as kpool,
            tc.tile_pool(name="gat", bufs=3) as gpool,
            tc.tile_pool(name="acc", bufs=2) as apool,
            tc.tile_pool(name="out", bufs=2) as opool,
        ):
            # grid is loaded per column-chunk (see CHUNKS below) so the
            # first chunk's coordinate math can start immediately.
            sb_grid = cpool.tile([128, PPX * 2], F32)

            # --- coordinate transform, replicating reference f32 op-for-op ---
            # x and y transforms are identical (H == W == 256), so run ONE
            # chain over the whole interleaved [128, 1024] grid tile and
            # deinterleave with stride-2 views afterwards.
            def coord_chain(g_view, eng, width):
                """g_view: [128, width] normalized coords. Returns (x1, um, fm, g):
                x1 = floor(snapped unnormalized coord); um = (x1+1-x)*[x1>=0];
                fm = (x - x1)*[x1<=254]; g = [x1>=0] (0/1 mask)."""
                ts, tt = eng.tensor_scalar, eng.tensor_tensor
                P = [128, width]
                x = kpool.tile(P, F32, tag=f"x{id(g_view)}")
                r = kpool.tile(P, F32, tag=f"r{id(g_view)}")
                m = kpool.tile(P, F32, tag=f"m{id(g_view)}")
                d = kpool.tile(P, F32, tag=f"d{id(g_view)}")
                x1 = kpool.tile(P, F32, tag=f"x1{id(g_view)}")
                um = kpool.tile(P, F32, tag=f"um{id(g_view)}")
                fm = kpool.tile(P, F32, tag=f"fm{id(g_view)}")
                g = kpool.tile(P, F32, tag=f"g{id(g_view)}")
                # x = ((g + 1) * 256 - 1) * 0.5   (same rounding sequence as ref)
                ts(out=x[:], in0=g_view, scalar1=1.0, scalar2=None,
                   op0=mybir.AluOpType.add)
                ts(out=x[:], in0=x[:], scalar1=256.0, scalar2=-1.0,
                   op0=mybir.AluOpType.mult, op1=mybir.AluOpType.add)
                ts(out=x[:], in0=x[:], scalar1=0.5, scalar2=None,
                   op0=mybir.AluOpType.mult)
                # snap-to-ceil: cc = ceil(x); if cc - x < 1e-5: x = cc
                ts(out=r[:], in0=x[:], scalar1=MAGIC, scalar2=-MAGIC,
                   op0=mybir.AluOpType.add, op1=mybir.AluOpType.add)
                tt(out=m[:], in0=r[:], in1=x[:], op=mybir.AluOpType.is_lt)
                tt(out=r[:], in0=r[:], in1=m[:], op=mybir.AluOpType.add)  # r = ceil
                tt(out=d[:], in0=r[:], in1=x[:], op=mybir.AluOpType.subtract)
                ts(out=m[:], in0=d[:], scalar1=1e-5, scalar2=None,
                   op0=mybir.AluOpType.is_lt)
                tt(out=d[:], in0=d[:], in1=m[:], op=mybir.AluOpType.mult)
                tt(out=x[:], in0=x[:], in1=d[:], op=mybir.AluOpType.add)
                # x1 = floor(x)
                ts(out=r[:], in0=x[:], scalar1=MAGIC, scalar2=-MAGIC,
                   op0=mybir.AluOpType.add, op1=mybir.AluOpType.add)
                tt(out=m[:], in0=r[:], in1=x[:], op=mybir.AluOpType.is_gt)
                tt(out=x1[:], in0=r[:], in1=m[:], op=mybir.AluOpType.subtract)
                # fm = (x - x1) * [x1 <= 254]
                tt(out=fm[:], in0=x[:], in1=x1[:], op=mybir.AluOpType.subtract)
                ts(out=m[:], in0=x1[:], scalar1=254.0, scalar2=None,
                   op0=mybir.AluOpType.is_le)
                tt(out=fm[:], in0=fm[:], in1=m[:], op=mybir.AluOpType.mult)
                # g = [x1 >= 0];  um = ((x1 + 1) - x) * g
                ts(out=r[:], in0=x1[:], scalar1=1.0, scalar2=None,
                   op0=mybir.AluOpType.add)
                tt(out=um[:], in0=r[:], in1=x[:], op=mybir.AluOpType.subtract)
                ts(out=g[:], in0=x1[:], scalar1=0.0, scalar2=None,
                   op0=mybir.AluOpType.is_ge)
                tt(out=um[:], in0=um[:], in1=g[:], op=mybir.AluOpType.mult)
                return x1, um, fm, g

            # Compute coords/weights/indices in two column chunks so the
            # first gathers can start ~20us earlier; the second (larger)
            # chunk's DVE math overlaps the gather stream, which has slack.
            CHUNKS = [(0, 2 * K), (2 * K, PPX)]

            def compute_meta(a, b):
                wc = b - a
                nc.sync.dma_start(out=sb_grid[:, 2 * a:2 * b],
                                  in_=grid_d[:, 2 * a:2 * b])
                c1, umc, fmc, gc = coord_chain(sb_grid[:, 2 * a:2 * b],
                                               nc.vector, 2 * wc)
                x1, umx, fmx, gx = (v[:, 0::2] for v in (c1, umc, fmc, gc))
                y1, umy, fmy, gy = (v[:, 1::2] for v in (c1, umc, fmc, gc))

                # per-slot weights of the clamped 2x2 block: bx = max(x1,0)
                # = x1*gx; slot c0 weight sxA = umx + fmx*(1-gx) (for x1==-1
                # the valid x2=0 tap lands on c0), slot c1 sxB = fmx*gx.
                P = [128, wc]
                tt = nc.vector.tensor_tensor
                ts = nc.vector.tensor_scalar
                sxA = kpool.tile(P, F32, tag=f"sxA{a}")
                sxB = kpool.tile(P, F32, tag=f"sxB{a}")
                syA = kpool.tile(P, F32, tag=f"syA{a}")
                syB = kpool.tile(P, F32, tag=f"syB{a}")
                tmp = kpool.tile(P, F32, tag=f"tmp{a}")
                tt(out=sxB[:], in0=fmx, in1=gx, op=mybir.AluOpType.mult)
                tt(out=tmp[:], in0=umx, in1=fmx, op=mybir.AluOpType.add)
                tt(out=sxA[:], in0=tmp[:], in1=sxB[:],
                   op=mybir.AluOpType.subtract)
                tt(out=syB[:], in0=fmy, in1=gy, op=mybir.AluOpType.mult)
                tt(out=tmp[:], in0=umy, in1=fmy, op=mybir.AluOpType.add)
                tt(out=syA[:], in0=tmp[:], in1=syB[:],
                   op=mybir.AluOpType.subtract)

                w00 = kpool.tile(P, F32, tag=f"w00{a}")
                w01 = kpool.tile(P, F32, tag=f"w01{a}")
                w10 = kpool.tile(P, F32, tag=f"w10{a}")
                w11 = kpool.tile(P, F32, tag=f"w11{a}")
                tt(out=w00[:], in0=syA[:], in1=sxA[:], op=mybir.AluOpType.mult)
                tt(out=w01[:], in0=syA[:], in1=sxB[:], op=mybir.AluOpType.mult)
                tt(out=w10[:], in0=syB[:], in1=sxA[:], op=mybir.AluOpType.mult)
                tt(out=w11[:], in0=syB[:], in1=sxB[:], op=mybir.AluOpType.mult)

                # block index: (y1*gy)*256 + x1*gx
                bxf = kpool.tile(P, F32, tag=f"bxf{a}")
                byf = kpool.tile(P, F32, tag=f"byf{a}")
                idx = kpool.tile(P, I32, tag=f"idx{a}")
                tt(out=bxf[:], in0=x1, in1=gx, op=mybir.AluOpType.mult)
                tt(out=byf[:], in0=y1, in1=gy, op=mybir.AluOpType.mult)
                ts(out=byf[:], in0=byf[:], scalar1=256.0, scalar2=None,
                   op0=mybir.AluOpType.mult)
                tt(out=byf[:], in0=byf[:], in1=bxf[:], op=mybir.AluOpType.add)
                nc.vector.tensor_copy(out=idx[:], in_=byf[:])
                return dict(a=a, b=b, idx=idx, w00=w00, w01=w01, w10=w10,
                            w11=w11)

            metas = [compute_meta(a, b) for a, b in CHUNKS]

            # --- gather + weighted sum, tile by tile -----------------------
            # HW indirect DMA consumes exactly ONE index per partition per
            # instruction (dst free dim is filled with consecutive table rows
            # from idx[p,0]), so each pixel-column is one 1KB-per-partition
            # gather instruction.
            for t in range(NT):
                mt = next(m for m in metas
                          if m["a"] <= t * K and (t + 1) * K <= m["b"])
                lo = t * K - mt["a"]
                idx = mt["idx"]
                G = gpool.tile([128, K * BLKC], F32)  # [p, j, slot(4), C]
                for j in range(K):
                    nc.gpsimd.indirect_dma_start(
                        out=G[:, j * BLKC:(j + 1) * BLKC],
                        out_offset=None,
                        in_=table[:],
                        in_offset=IndirectOffsetOnAxis(
                            ap=idx[:, lo + j:lo + j + 1], axis=0),
                    )
                Gv = G[:].rearrange("p (j e) -> p j e", j=K)  # [128, K, 256]
                s00 = Gv[:, :, 0:C]
                s01 = Gv[:, :, C:2 * C]
                s10 = Gv[:, :, 2 * C:3 * C]
                s11 = Gv[:, :, 3 * C:4 * C]
                sl = slice(lo, lo + K)
                w00b = mt["w00"][:, sl].to_broadcast([128, K, C])
                w01b = mt["w01"][:, sl].to_broadcast([128, K, C])
                w10b = mt["w10"][:, sl].to_broadcast([128, K, C])
                w11b = mt["w11"][:, sl].to_broadcast([128, K, C])

                t1 = apool.tile([128, K * C], F32, tag="t1")
                t2 = apool.tile([128, K * C], F32, tag="t2")
                O = opool.tile([128, K * C], F32)
                vtt = nc.vector.tensor_tensor
                vtt(out=t1[:], in0=s00, in1=w00b, op=mybir.AluOpType.mult)
                vtt(out=t2[:], in0=s01, in1=w01b, op=mybir.AluOpType.mult)
                vtt(out=t1[:], in0=t1[:], in1=t2[:], op=mybir.AluOpType.add)
                vtt(out=t2[:], in0=s10, in1=w10b, op=mybir.AluOpType.mult)
                vtt(out=O[:], in0=s11, in1=w11b, op=mybir.AluOpType.mult)
                vtt(out=t2[:], in0=t2[:], in1=O[:], op=mybir.AluOpType.add)
                vtt(out=O[:], in0=t1[:], in1=t2[:], op=mybir.AluOpType.add)
                nc.sync.dma_start(out=out_d[:, t * K * C:(t + 1) * K * C],
                                  in_=O[:])
    return nc


def _get_program():
    if "nc" not in _CACHE:
        _CACHE["nc"] = _build_program()
    return _CACHE["nc"]


def _make_in_maps(inputs, grid):
    in_maps = []
    P = np.zeros((H + 1, W + 1, C), dtype=np.float32)
    for i in range(N):
        P[:H, :W] = inputs[i]
        blk = np.empty((H, W, 4, C), dtype=np.float32)
        blk[:, :, 0] = P[:H, :W]
        blk[:, :, 1] = P[:H, 1:W + 1]
        blk[:, :, 2] = P[1:H + 1, :W]
        blk[:, :, 3] = P[1:H + 1, 1:W + 1]
        g = np.ascontiguousarray(grid[i], dtype=np.float32).reshape(128, PPX * 2)
        in_maps.append({"table": blk.reshape(NBLK, BLKC), "grid": g})
    return in_maps


def run(inputs, grid, trace=False, **kw):
    nc = _get_program()
    in_maps = _make_in_maps(inputs, grid)
    res = run_bass_kernel_spmd(nc, in_maps, list(range(N)), trace=trace, **kw)
    out = np.empty((N, H, W, C), dtype=np.float32)
    for i in range(N):
        out[i] = res.results[i]["out"].reshape(H, W, C)
    return out, res


def kernel(inputs, grid):
    out, _ = run(inputs, grid, trace=False)
    return out
